# revision 1
# baseline (speedup 1.0000x reference)
"""Multi-head causal attention (B=4, T=2048, C=1024, H=16) on 8 TRN2 cores.

Sharding: core i handles batch b = i//2 and head-group g = i%2 (8 heads each).
Each core computes qkv projection for its heads, causal attention, and a
partial output projection (its heads' rows of W_o). The host sums the two
partials per batch and adds b_o.

Device kernel (per core, same SPMD program), all matmuls bf16 with fp32 PSUM:
  - qkT = (Wqk^T x^T) computed transposed: [1024 feats, 2048], bf16
  - v   = x Wv computed natural:           [2048, 512],        bf16
  - per head pair (2hp, 2hp+1), per 512-wide query block, per 128-wide key
    chunk (causal-trimmed):
      S^T = K^T q^T   [128 tk, tq]  (row-packed pairs, K=64 contraction)
      P^T = exp(S^T * 0.125)        (ScalarE, diagonal chunks masked on DVE)
      AV^T += [V | 1]^T P^T         [65, tq]  (row 64 = softmax denominator)
    scores for the next query block are chunk-interleaved with AV of the
    current one so ScalarE (exp) stays saturated; softmax normalization
    (reciprocal + ones-outer-product broadcast + divide) lags 2 segments
  - out_part = attT^T W_o rows      [2048, 1024], emitted per query-block
    group as soon as its last head pair normalizes; stored bf16, host sums
    the two per-batch partials in fp32 and adds b_o.
"""

import sys

sys.path.insert(0, "/opt/trn_rl_repo")

import numpy as np
import ml_dtypes

BF16 = ml_dtypes.bfloat16

B, T, C, H, D = 4, 2048, 1024, 16, 64
HPC = 8        # heads per core
CQ = HPC * D   # 512
NCORES = 8
P = 128


def _split_waits(nc):
    """This container's walrus accepts only ONE sync wait per instruction.
    Split any instruction carrying N>1 waits into N-1 single-wait NoOps on
    the same engine immediately before it."""
    import concourse.mybir as mybir

    ctr = 0
    for fn in nc.m.functions:
        for bb in fn.blocks:
            insts = list(bb.instructions)
            new_insts = []
            changed = False
            for inst in insts:
                si = inst.sync_info
                if si is not None and si.on_wait and len(si.on_wait) > 1:
                    waits = list(si.on_wait)
                    for w in waits[:-1]:
                        ctr += 1
                        nop = mybir.InstNoOp(
                            name=f"I-wsplit-{ctr}",
                            engine=inst.engine,
                            ins=[],
                            outs=[],
                            sync_info=mybir.SyncInfo(on_wait=[w], on_update=[]),
                        )
                        new_insts.append(nop)
                    si.on_wait = [waits[-1]]
                    changed = True
                new_insts.append(inst)
            if changed:
                bb.instructions[:] = new_insts
    return ctr


def _declare(nc):
    import concourse.mybir as mybir

    bf = mybir.dt.bfloat16
    f32 = mybir.dt.float32
    return dict(
        xT=nc.dram_tensor("xT", [C, T], bf, kind="ExternalInput").ap(),
        wqk=nc.dram_tensor("wqk", [C, 2 * CQ], bf, kind="ExternalInput").ap(),
        bqk=nc.dram_tensor("bqk", [P, 8], f32, kind="ExternalInput").ap(),
        wv=nc.dram_tensor("wv", [C, CQ], bf, kind="ExternalInput").ap(),
        bvb=nc.dram_tensor("bvb", [P, CQ], f32, kind="ExternalInput").ap(),
        wo=nc.dram_tensor("wo", [CQ, C], bf, kind="ExternalInput").ap(),
        maskT=nc.dram_tensor("maskT", [P, P], bf, kind="ExternalInput").ap(),
        ones1=nc.dram_tensor("ones1", [1, 64], bf, kind="ExternalInput").ap(),
        outp=nc.dram_tensor("outp", [T, C], bf, kind="ExternalOutput").ap(),
    )


def _emit(nc, tc, aps):
    import concourse.mybir as mybir
    from concourse.alu_op_type import AluOpType

    bf = mybir.dt.bfloat16
    f32 = mybir.dt.float32
    Exp = mybir.ActivationFunctionType.Exp

    xT = aps["xT"]; wqk = aps["wqk"]; bqk = aps["bqk"]; wv = aps["wv"]
    bvb = aps["bvb"]; wo = aps["wo"]; maskT = aps["maskT"]
    ones1 = aps["ones1"]; outp = aps["outp"]

    NTT = T // 512  # 4 query blocks
    VW = HPC * 65   # 520: v row layout (64 cols + ones col per head)

    with tc.tile_pool(name="const", bufs=1) as cpool:
        wo_sb = cpool.tile([P, 4 * 1024], bf)
        maskT_sb = cpool.tile([P, P], bf)
        ones1_sb = cpool.tile([1, 64], bf)
        qkT_sb = cpool.tile([P, 8 * T], bf)
        v_sb = cpool.tile([P, 16 * VW], bf)
        attn_sb = cpool.tile([P, 16 * 512], bf)     # unnorm AV^T, pair layout
        sums_e = cpool.tile([1, 16 * 512], f32)     # even-head denominators
        sums_o = cpool.tile([1, 16 * 512], f32)     # odd-head denominators

        for hc in range(4):
            nc.sync.dma_start(wo_sb[:, hc * 1024:(hc + 1) * 1024], wo[hc * P:(hc + 1) * P, :])
        nc.sync.dma_start(maskT_sb[:], maskT[:])
        nc.sync.dma_start(ones1_sb[:], ones1[:])

        # ones columns of v (col 64 of each head's 65-wide strip)
        v_ones = v_sb.rearrange("p (a c) -> p a c", c=65)
        nc.vector.memset(v_ones[:, :, 64:65], 1.0)

        # ---------------- qkv projection ----------------
        # (projection-only inputs live in a pool that closes afterwards, so
        # the attention pool below reuses their SBUF space)
        with tc.tile_pool(name="proj", bufs=1) as ppool, \
             tc.tile_pool(name="ps1", bufs=1, space="PSUM") as ps1:
            xT_sb = ppool.tile([P, 8 * T], bf)
            wqk_sb = ppool.tile([P, 8 * 1024], bf)
            wv_sb = ppool.tile([P, 8 * CQ], bf)
            bqk_sb = ppool.tile([P, 8], f32)
            bvb_sb = ppool.tile([P, CQ], f32)
            for cc in range(8):
                # halves per chunk: PE's first accumulation can start sooner
                for h in range(2):
                    nc.sync.dma_start(
                        xT_sb[:, cc * T + h * (T // 2): cc * T + (h + 1) * (T // 2)],
                        xT[cc * P:(cc + 1) * P, h * (T // 2):(h + 1) * (T // 2)])
                nc.sync.dma_start(wqk_sb[:, cc * 1024:(cc + 1) * 1024], wqk[cc * P:(cc + 1) * P, :])
            for cc in range(8):
                nc.sync.dma_start(wv_sb[:, cc * CQ:(cc + 1) * CQ], wv[cc * P:(cc + 1) * P, :])
            nc.sync.dma_start(bqk_sb[:], bqk[:])
            nc.sync.dma_start(bvb_sb[:], bvb[:])

            # q^T / k^T : weights stationary, xT moving -> qkT [feat, t]
            for nt in range(8):
                psqs = [
                    ps1.tile([P, 512], f32, tag="psq", bufs=6, name=f"psq_{nt}_{tt}")
                    for tt in range(NTT)
                ]
                for cc in range(8):
                    for tt in range(NTT):
                        nc.tensor.matmul(
                            psqs[tt][:],
                            wqk_sb[:, cc * 1024 + nt * P: cc * 1024 + (nt + 1) * P],
                            xT_sb[:, cc * T + tt * 512: cc * T + (tt + 1) * 512],
                            start=(cc == 0),
                            stop=(cc == 7),
                        )
                for tt in range(NTT):
                    nc.vector.tensor_scalar(
                        qkT_sb[:, nt * T + tt * 512: nt * T + (tt + 1) * 512],
                        psqs[tt][:],
                        bqk_sb[:, nt:nt + 1],
                        None,
                        op0=AluOpType.add,
                    )
            # v natural: xT stationary, wv moving -> v [t, feat]
            for tt16 in range(16):
                psv = ps1.tile([P, CQ], f32, tag="psv", bufs=2, name=f"psv_{tt16}")
                for cc in range(8):
                    nc.tensor.matmul(
                        psv[:],
                        xT_sb[:, cc * T + tt16 * P: cc * T + (tt16 + 1) * P],
                        wv_sb[:, cc * CQ:(cc + 1) * CQ],
                        start=(cc == 0),
                        stop=(cc == 7),
                    )
                vv = v_sb[:, tt16 * VW:(tt16 + 1) * VW].rearrange(
                    "p (a c) -> p a c", c=65
                )
                nc.vector.tensor_tensor(
                    vv[:, :, 0:64],
                    psv[:].rearrange("p (a c) -> p a c", c=64),
                    bvb_sb[:].rearrange("p (a c) -> p a c", c=64),
                    op=AluOpType.add,
                )

        # ---------------- attention ----------------
        # Per head pair: scores bursts (PE) run one query-block ahead of AV
        # bursts, chunk-interleaved, so ACT (exp) stays saturated while PE
        # fills the gaps with AV matmuls. Normalization (reciprocal, R
        # broadcast, divide) is emitted with a 2-segment lag so its small-op
        # chains never stall PE.
        with tc.tile_pool(name="ps2", bufs=1, space="PSUM") as ps2, \
             tc.tile_pool(name="ps4", bufs=1, space="PSUM") as ps4, \
             tc.tile_pool(name="att", bufs=1) as apool:
            pts = {}

            def s_chunk(hp, tb, j):
                h0, h1 = 2 * hp, 2 * hp + 1
                off = j * P - tb * 512
                nstart = max(off, 0)
                pss = ps2.tile([P, 1024], f32, tag="pss", bufs=2, name=f"pss_{hp}_{tb}_{j}")
                pt = apool.tile([P, 1024], bf, tag="pt", bufs=30, name=f"pt_{hp}_{tb}_{j}")
                pts[(hp, tb, j)] = pt
                for i, hl in enumerate((h0, h1)):
                    base = (hl % 2) * 64
                    nc.tensor.matmul(
                        pss[:, i * 512 + nstart: i * 512 + 512],
                        qkT_sb[base:base + 64,
                               (4 + hp) * T + j * P: (4 + hp) * T + (j + 1) * P],
                        qkT_sb[base:base + 64,
                               hp * T + tb * 512 + nstart: hp * T + (tb + 1) * 512],
                        start=True,
                        stop=True,
                    )
                pw = pss.rearrange("p (a c) -> p a c", c=512)
                ptw = pt.rearrange("p (a c) -> p a c", c=512)
                nc.scalar.activation(
                    ptw[:, :, nstart:512], pw[:, :, nstart:512], Exp, scale=0.125,
                )
                if off >= 0:
                    for i in range(2):
                        nc.vector.tensor_tensor(
                            pt[:, i * 512 + nstart: i * 512 + nstart + P],
                            pt[:, i * 512 + nstart: i * 512 + nstart + P],
                            maskT_sb[:],
                            op=AluOpType.mult,
                        )

            def av_chunk(hp, tb, j, psav):
                h0, h1 = 2 * hp, 2 * hp + 1
                jmax = 4 * tb + 3
                off = j * P - tb * 512
                nstart = max(off, 0)
                for i, hl in enumerate((h0, h1)):
                    nc.tensor.matmul(
                        psav[i][0:65, nstart:512],
                        v_sb[:, j * VW + hl * 65: j * VW + (hl + 1) * 65],
                        pts[(hp, tb, j)][:, i * 512 + nstart: i * 512 + 512],
                        start=(j == 0),
                        stop=(j == jmax),
                    )
                if j == jmax:
                    seg = hp * NTT + tb
                    nc.vector.tensor_copy(attn_sb[0:64, seg * 512:(seg + 1) * 512], psav[0][0:64, :])
                    nc.vector.tensor_copy(attn_sb[64:128, seg * 512:(seg + 1) * 512], psav[1][0:64, :])
                    nc.vector.tensor_copy(sums_e[0:1, seg * 512:(seg + 1) * 512], psav[0][64:65, :])
                    nc.vector.tensor_copy(sums_o[0:1, seg * 512:(seg + 1) * 512], psav[1][64:65, :])

            def norm_seg(seg):
                sl = slice(seg * 512, (seg + 1) * 512)
                rec_e = apool.tile([1, 512], bf, tag="rece", bufs=2, name=f"rece_{seg}")
                rec_o = apool.tile([1, 512], bf, tag="reco", bufs=2, name=f"reco_{seg}")
                with nc.allow_low_precision(reason="bf16 softmax denominators"):
                    nc.vector.reciprocal(rec_e[:], sums_e[0:1, sl])
                    nc.vector.reciprocal(rec_o[:], sums_o[0:1, sl])
                psr = ps4.tile([P, 512], f32, tag="psr", bufs=2, name=f"psr_{seg}")
                rsb = apool.tile([P, 512], bf, tag="rsb", bufs=2, name=f"rsb_{seg}")
                nc.tensor.matmul(psr[0:64, :], ones1_sb[:], rec_e[0:1, :], start=True, stop=True)
                nc.tensor.matmul(psr[64:128, :], ones1_sb[:], rec_o[0:1, :], start=True, stop=True)
                nc.vector.tensor_copy(rsb[:], psr[:])
                nc.vector.tensor_tensor(attn_sb[0:64, sl], attn_sb[0:64, sl], rsb[0:64, :], op=AluOpType.mult)
                nc.vector.tensor_tensor(attn_sb[64:128, sl], attn_sb[64:128, sl], rsb[64:128, :], op=AluOpType.mult)

            def oproj_group(tbg, opool):
                # output projection for query rows tbg*512 .. +512 (4 t-tiles)
                for tt16 in range(tbg * 4, tbg * 4 + 4):
                    psos = [
                        ps4.tile([P, 512], f32, tag="psr", bufs=2, name=f"pso_{tt16}_{mb}")
                        for mb in range(2)
                    ]
                    for hc in range(4):
                        seg = hc * NTT + tt16 // 4
                        col = (seg * 4 + tt16 % 4) * P
                        for mb in range(2):
                            nc.tensor.matmul(
                                psos[mb][:],
                                attn_sb[:, col: col + P],
                                wo_sb[:, hc * 1024 + mb * 512: hc * 1024 + (mb + 1) * 512],
                                start=(hc == 0),
                                stop=(hc == 3),
                            )
                    for mb in range(2):
                        osb = opool.tile([P, 512], bf, tag="osb", bufs=4, name=f"osb_{tt16}_{mb}")
                        nc.scalar.copy(osb[:], psos[mb][:])
                        nc.sync.dma_start(
                            outp[tt16 * P:(tt16 + 1) * P, mb * 512:(mb + 1) * 512],
                            osb[:],
                        )

            with tc.tile_pool(name="out", bufs=1) as opool:
                # flattened segment pipeline: S-burst of seg+1 interleaves with
                # AV of seg (across head-pair boundaries); normalization lags 2
                # segments; the output projection for a query-block group is
                # emitted as soon as its last head-pair segment is normalized.
                def seg_hp_tb(s):
                    # tb-major order: all head pairs finish query-block tb
                    # before tb+1, so each o-proj group fires early and
                    # overlaps subsequent attention instead of bunching at
                    # the tail.
                    return s % 4, s // 4

                norm_pending = []
                normed = set()

                def flush_norm():
                    seg, hp, tb = norm_pending.pop(0)
                    norm_seg(seg)
                    normed.add(seg)
                    if all((h * NTT + tb) in normed for h in range(4)):
                        oproj_group(tb, opool)

                for j in range(4):
                    s_chunk(0, 0, j)
                for s in range(16):
                    hp, tb = seg_hp_tb(s)
                    psav = [
                        ps2.tile([P, 512], f32, tag="psav", bufs=2, name=f"psav_{hp}_{tb}_{i}")
                        for i in range(2)
                    ]
                    js_a = list(range(4 * tb + 4))
                    if s + 1 < 16:
                        nhp, ntb = seg_hp_tb(s + 1)
                        js_s = list(range(4 * ntb + 4))
                    else:
                        js_s = []
                    for m in range(max(len(js_a), len(js_s))):
                        if m < len(js_s):
                            s_chunk(nhp, ntb, js_s[m])
                        if m < len(js_a):
                            av_chunk(hp, tb, js_a[m], psav)
                    norm_pending.append((hp * NTT + tb, hp, tb))
                    if len(norm_pending) > 2:
                        flush_norm()
                while norm_pending:
                    flush_norm()


_cached = {}


def build_program(split=True, ncopies=1):
    key = ("nc", ncopies)
    if key not in _cached:
        import concourse.bass as bass
        import concourse.tile as tile

        nc = bass.Bass("TRN2", target_bir_lowering=False, debug=False)
        with tile.TileContext(nc) as tc:
            aps = _declare(nc)
            for _ in range(ncopies):
                _emit(nc, tc, aps)
        _cached[key] = nc
    if split and not _cached.get(("split", ncopies)):
        _split_waits(_cached[key])
        _cached[("split", ncopies)] = True
    return _cached[key]


def make_in_maps(x, W_qkv, b_qkv, W_o):
    x = np.asarray(x, dtype=np.float32)
    W_qkv = np.asarray(W_qkv, dtype=np.float32)
    b_qkv = np.asarray(b_qkv, dtype=np.float32)
    W_o = np.asarray(W_o, dtype=np.float32)
    maskT = np.triu(np.ones((P, P), np.float32)).astype(BF16)
    ones1 = np.ones((1, 64), BF16)
    in_maps = []
    for core in range(NCORES):
        b, g = core // 2, core % 2
        qs = slice(g * CQ, (g + 1) * CQ)
        xTc = np.ascontiguousarray(x[b].T).astype(BF16)
        wq = W_qkv[:, 0:C][:, qs]
        wk = W_qkv[:, C:2 * C][:, qs]
        wvs = np.ascontiguousarray(W_qkv[:, 2 * C:3 * C][:, qs]).astype(BF16)
        wqks = np.ascontiguousarray(np.concatenate([wq, wk], axis=1)).astype(BF16)
        bq = b_qkv[0:C][qs]
        bk = b_qkv[C:2 * C][qs]
        bv = b_qkv[2 * C:3 * C][qs]
        bqk_t = np.ascontiguousarray(
            np.concatenate([bq, bk]).reshape(8, P).T
        ).astype(np.float32)
        bvb = np.ascontiguousarray(
            np.broadcast_to(bv, (P, CQ))
        ).astype(np.float32)
        wos = np.ascontiguousarray(W_o[qs, :]).astype(BF16)
        in_maps.append(
            dict(xT=xTc, wqk=wqks, bqk=bqk_t, wv=wvs, bvb=bvb, wo=wos,
                 maskT=maskT, ones1=ones1)
        )
    return in_maps


def run(x, W_qkv, b_qkv, W_o, b_o, trace=False, trace_kwargs=None):
    import time as _time

    from concourse.bass_utils import run_bass_kernel_spmd

    nc = build_program()
    in_maps = make_in_maps(x, W_qkv, b_qkv, W_o)
    last_err = None
    for attempt in range(3):
        try:
            res = run_bass_kernel_spmd(
                nc, in_maps, core_ids=list(range(NCORES)), trace=trace,
                **(trace_kwargs or {}),
            )
            break
        except Exception as e:  # transient device wedge -> retry
            last_err = e
            _time.sleep(5)
    else:
        raise last_err
    b_o = np.asarray(b_o, dtype=np.float32)
    out = np.empty((B, T, C), np.float32)
    for b in range(B):
        out[b] = (res.results[2 * b]["outp"].astype(np.float32)
                  + res.results[2 * b + 1]["outp"].astype(np.float32) + b_o)
    return out, res


def kernel(x, W_qkv, b_qkv, W_o, b_o):
    out, _ = run(x, W_qkv, b_qkv, W_o, b_o, trace=False)
    return out



# revision 6
# speedup vs baseline: 2.5255x; 2.5255x over previous
"""Multi-head causal attention (B=4, T=2048, C=1024, H=16) on 8 TRN2 cores.

Sharding: core i handles batch b = i//2 and head-group g = i%2 (8 heads each).
Each core computes qkv projection for its heads, causal attention, and a
partial output projection (its heads' rows of W_o). The host sums the two
partials per batch and adds b_o.

Device kernel (per core, same SPMD program):
  - q^T/k^T projection in fp8(e4m3) DoubleRow matmuls (2 contraction chunks
    per instruction): qkT = (64*Wqk)^T x^T stored bf16 at 64x scale; the 64^2
    factor is folded into the exp scale. fp8 on this path is softmax-damped
    (abs score error ~1e-2 -> p error ~1%).
  - v = x Wv natural in bf16:            [2048, 512]
  - per head pair (2hp, 2hp+1), per 512-wide query block, per 128-wide key
    chunk (causal-trimmed):
      S^T = K^T q^T   [128 tk, tq]  (row-packed pairs, K=64 contraction)
      P^T = exp(S^T * 0.125/4096)   (ScalarE; diagonal chunks masked on DVE)
      AV^T += [V | 1]^T P^T         [65, tq]  (row 64 = softmax denominator)
    the S stream runs one segment plus SHIFT chunks ahead of the AV stream so
    ScalarE (exp) stays saturated and psum drains have cover.
  - psum drains on DVE (GPSIMD cannot touch PSUM); softmax normalization:
    DVE reciprocal, PE ones-outer-product broadcast, DVE in-place multiply.
  - out_part = attT^T W_o rows [2048, 1024]: per-tt16 units (8 matmuls)
    spread into the attention chunk stream; host sums the two per-batch
    partials in fp32 and adds b_o.
"""

import sys

sys.path.insert(0, "/opt/trn_rl_repo")

import numpy as np
import ml_dtypes

BF16 = ml_dtypes.bfloat16
E4M3 = ml_dtypes.float8_e4m3fn

B, T, C, H, D = 4, 2048, 1024, 16, 64
HPC = 8        # heads per core
CQ = HPC * D   # 512
NCORES = 8
P = 128
WSCALE = 64.0  # fp8 weight pre-scale for the q/k projection
EXP_SCALE = 0.125 / (WSCALE * WSCALE)
SHIFT = 4      # AV stream lag (chunks) behind the S stream within a segment


def _split_waits(nc):
    """This container's walrus accepts only ONE sync wait per instruction.
    Split any instruction carrying N>1 waits into N-1 single-wait NoOps on
    the same engine immediately before it."""
    import concourse.mybir as mybir

    ctr = 0
    for fn in nc.m.functions:
        for bb in fn.blocks:
            insts = list(bb.instructions)
            new_insts = []
            changed = False
            for inst in insts:
                si = inst.sync_info
                if si is not None and si.on_wait and len(si.on_wait) > 1:
                    waits = list(si.on_wait)
                    for w in waits[:-1]:
                        ctr += 1
                        nop = mybir.InstNoOp(
                            name=f"I-wsplit-{ctr}",
                            engine=inst.engine,
                            ins=[],
                            outs=[],
                            sync_info=mybir.SyncInfo(on_wait=[w], on_update=[]),
                        )
                        new_insts.append(nop)
                    si.on_wait = [waits[-1]]
                    changed = True
                new_insts.append(inst)
            if changed:
                bb.instructions[:] = new_insts
    return ctr


def _declare(nc):
    import concourse.mybir as mybir

    bf = mybir.dt.bfloat16
    f8 = mybir.dt.float8e4
    f32 = mybir.dt.float32
    return dict(
        xT=nc.dram_tensor("xT", [C, T], bf, kind="ExternalInput").ap(),
        xT8=nc.dram_tensor("xT8", [C, T], f8, kind="ExternalInput").ap(),
        wqk8=nc.dram_tensor("wqk8", [C, 2 * CQ], f8, kind="ExternalInput").ap(),
        bqk=nc.dram_tensor("bqk", [P, 8], f32, kind="ExternalInput").ap(),
        wv=nc.dram_tensor("wv", [C, CQ], bf, kind="ExternalInput").ap(),
        bvb=nc.dram_tensor("bvb", [P, CQ], f32, kind="ExternalInput").ap(),
        wo=nc.dram_tensor("wo", [CQ, C], bf, kind="ExternalInput").ap(),
        maskT=nc.dram_tensor("maskT", [P, P], bf, kind="ExternalInput").ap(),
        ones1=nc.dram_tensor("ones1", [1, 64], bf, kind="ExternalInput").ap(),
        outp=nc.dram_tensor("outp", [T, C], bf, kind="ExternalOutput").ap(),
    )


def _emit(nc, tc, aps):
    import concourse.mybir as mybir
    from concourse.alu_op_type import AluOpType

    bf = mybir.dt.bfloat16
    f8 = mybir.dt.float8e4
    f32 = mybir.dt.float32
    Exp = mybir.ActivationFunctionType.Exp
    DR = mybir.MatmulPerfMode.DoubleRow

    xT = aps["xT"]; xT8 = aps["xT8"]; wqk8 = aps["wqk8"]; bqk = aps["bqk"]
    wv = aps["wv"]; bvb = aps["bvb"]; wo = aps["wo"]; maskT = aps["maskT"]
    ones1 = aps["ones1"]; outp = aps["outp"]

    NTT = T // 512  # 4 query blocks
    VW = HPC * 65   # 520: v row layout (64 cols + ones col per head)

    with tc.tile_pool(name="const", bufs=1) as cpool:
        wo_sb = cpool.tile([P, 4 * 1024], bf)
        maskT_sb = cpool.tile([P, P], bf)
        ones1_sb = cpool.tile([1, 64], bf)
        qkT_sb = cpool.tile([P, 8 * T], bf)
        v_sb = cpool.tile([P, 16 * VW], bf)
        attn_sb = cpool.tile([P, 16 * 512], bf)     # unnorm AV^T, pair layout
        # softmax denominators: partition 0 = even heads, partition 32 = odd
        # (engine partition bases must be 32-aligned)
        sums = cpool.tile([33, 16 * 512], bf)

        # ---------------- qkv projection ----------------
        with tc.tile_pool(name="proj", bufs=1) as ppool, \
             tc.tile_pool(name="ps1", bufs=1, space="PSUM") as ps1:
            xT8_sb = ppool.tile([P, 8 * T], f8)
            wqk8_sb = ppool.tile([P, 8 * 1024], f8)
            xT_sb = ppool.tile([P, 8 * T], bf)
            wv_sb = ppool.tile([P, 8 * CQ], bf)
            bqk_sb = ppool.tile([P, 8], f32)
            bvb_sb = ppool.tile([P, CQ], f32)

            # DMA order = consumption order: qk weights (split), then xT8 by
            # tt halves, then the v-projection inputs, then constants.
            for cc in range(8):
                for hh in range(2):
                    nc.sync.dma_start(
                        wqk8_sb[:, cc * 1024 + hh * 512: cc * 1024 + (hh + 1) * 512],
                        wqk8[cc * P:(cc + 1) * P, hh * 512:(hh + 1) * 512])
            nc.sync.dma_start(bqk_sb[:], bqk[:])
            for th in range(2):
                for cc in range(8):
                    nc.sync.dma_start(
                        xT8_sb[:, cc * T + th * 1024: cc * T + (th + 1) * 1024],
                        xT8[cc * P:(cc + 1) * P, th * 1024:(th + 1) * 1024])
            for cc in range(8):
                for hh in range(2):
                    nc.sync.dma_start(
                        xT_sb[:, cc * T + hh * (T // 2): cc * T + (hh + 1) * (T // 2)],
                        xT[cc * P:(cc + 1) * P, hh * (T // 2):(hh + 1) * (T // 2)])
            for cc in range(8):
                nc.sync.dma_start(wv_sb[:, cc * CQ:(cc + 1) * CQ], wv[cc * P:(cc + 1) * P, :])
            nc.sync.dma_start(bvb_sb[:], bvb[:])
            for hc in range(4):
                nc.sync.dma_start(wo_sb[:, hc * 1024:(hc + 1) * 1024], wo[hc * P:(hc + 1) * P, :])
            nc.sync.dma_start(maskT_sb[:], maskT[:])
            nc.sync.dma_start(ones1_sb[:], ones1[:])

            # ones columns of v (col 64 of each head's 65-wide strip)
            v_ones = v_sb.rearrange("p (a c) -> p a c", c=65)
            nc.vector.memset(v_ones[:, :, 64:65], 1.0)

            # q^T / k^T : fp8 DoubleRow, 2 contraction chunks per matmul.
            x8r = xT8_sb.rearrange("p (c t) -> p c t", t=T)
            w8r = wqk8_sb.rearrange("p (c n) -> p c n", n=1024)
            for tt in range(NTT):
                for nt in range(8):
                    psq = ps1.tile([P, 512], f32, tag="psq", bufs=3,
                                   name=f"psq_{tt}_{nt}")
                    for ccp in range(4):
                        nc.tensor.matmul(
                            psq[:],
                            w8r[:, 2 * ccp:2 * ccp + 2, nt * P:(nt + 1) * P],
                            x8r[:, 2 * ccp:2 * ccp + 2, tt * 512:(tt + 1) * 512],
                            start=(ccp == 0),
                            stop=(ccp == 3),
                            perf_mode=DR,
                        )
                    nc.vector.tensor_scalar(
                        qkT_sb[:, nt * T + tt * 512: nt * T + (tt + 1) * 512],
                        psq[:],
                        bqk_sb[:, nt:nt + 1],
                        None,
                        op0=AluOpType.add,
                    )
            # v natural: xT stationary, wv moving -> v [t, feat]
            for tt16 in range(16):
                psv = ps1.tile([P, CQ], f32, tag="psv", bufs=2, name=f"psv_{tt16}")
                for cc in range(8):
                    nc.tensor.matmul(
                        psv[:],
                        xT_sb[:, cc * T + tt16 * P: cc * T + (tt16 + 1) * P],
                        wv_sb[:, cc * CQ:(cc + 1) * CQ],
                        start=(cc == 0),
                        stop=(cc == 7),
                    )
                vv = v_sb[:, tt16 * VW:(tt16 + 1) * VW].rearrange(
                    "p (a c) -> p a c", c=65
                )
                nc.vector.tensor_tensor(
                    vv[:, :, 0:64],
                    psv[:].rearrange("p (a c) -> p a c", c=64),
                    bvb_sb[:].rearrange("p (a c) -> p a c", c=64),
                    op=AluOpType.add,
                )

        # ---------------- attention ----------------
        with tc.tile_pool(name="ps2", bufs=1, space="PSUM") as ps2, \
             tc.tile_pool(name="ps4", bufs=1, space="PSUM") as ps4, \
             tc.tile_pool(name="att", bufs=1) as apool:
            pts = {}

            def s_chunk(hp, tb, j):
                h0, h1 = 2 * hp, 2 * hp + 1
                off = j * P - tb * 512
                nstart = max(off, 0)
                pss = ps2.tile([P, 1024], f32, tag="pss", bufs=2, name=f"pss_{hp}_{tb}_{j}")
                pt = apool.tile([P, 1024], bf, tag="pt", bufs=30, name=f"pt_{hp}_{tb}_{j}")
                pts[(hp, tb, j)] = pt
                for i, hl in enumerate((h0, h1)):
                    base = (hl % 2) * 64
                    nc.tensor.matmul(
                        pss[:, i * 512 + nstart: i * 512 + 512],
                        qkT_sb[base:base + 64,
                               (4 + hp) * T + j * P: (4 + hp) * T + (j + 1) * P],
                        qkT_sb[base:base + 64,
                               hp * T + tb * 512 + nstart: hp * T + (tb + 1) * 512],
                        start=True,
                        stop=True,
                    )
                pw = pss.rearrange("p (a c) -> p a c", c=512)
                ptw = pt.rearrange("p (a c) -> p a c", c=512)
                nc.scalar.activation(
                    ptw[:, :, nstart:512], pw[:, :, nstart:512], Exp, scale=EXP_SCALE,
                )
                if off >= 0:
                    for i in range(2):
                        nc.vector.tensor_tensor(
                            pt[:, i * 512 + nstart: i * 512 + nstart + P],
                            pt[:, i * 512 + nstart: i * 512 + nstart + P],
                            maskT_sb[:],
                            op=AluOpType.mult,
                        )

            def av_chunk(hp, tb, j, psav):
                h0, h1 = 2 * hp, 2 * hp + 1
                jmax = 4 * tb + 3
                off = j * P - tb * 512
                nstart = max(off, 0)
                for i, hl in enumerate((h0, h1)):
                    nc.tensor.matmul(
                        psav[i][0:65, nstart:512],
                        v_sb[:, j * VW + hl * 65: j * VW + (hl + 1) * 65],
                        pts[(hp, tb, j)][:, i * 512 + nstart: i * 512 + 512],
                        start=(j == 0),
                        stop=(j == jmax),
                    )
                if j == jmax:
                    seg = hp * NTT + tb
                    sl = slice(seg * 512, (seg + 1) * 512)
                    nc.vector.tensor_copy(attn_sb[0:64, sl], psav[0][0:64, :])
                    nc.vector.tensor_copy(attn_sb[64:128, sl], psav[1][0:64, :])
                    with nc.allow_low_precision(reason="bf16 softmax denominators"):
                        nc.vector.tensor_copy(sums[0:1, sl], psav[0][64:65, :])
                        nc.vector.tensor_copy(sums[32:33, sl], psav[1][64:65, :])

            def norm_seg(seg, tail):
                sl = slice(seg * 512, (seg + 1) * 512)
                rec_e = apool.tile([1, 512], bf, tag="rece", bufs=2, name=f"rece_{seg}")
                rec_o = apool.tile([1, 512], bf, tag="reco", bufs=2, name=f"reco_{seg}")
                with nc.allow_low_precision(reason="bf16 softmax denominators"):
                    nc.vector.reciprocal(rec_e[:], sums[0:1, sl])
                    nc.vector.reciprocal(rec_o[:], sums[32:33, sl])
                rsb = apool.tile([P, 512], bf, tag="rsb", bufs=2, name=f"rsb_{seg}")
                # PE ones-outer-product broadcast (partition-stride-0 APs are
                # rejected by both DVE and DMA lowering)
                psr = ps4.tile([P, 512], f32, tag="pso", bufs=2, name=f"psr_{seg}")
                nc.tensor.matmul(psr[0:64, :], ones1_sb[:], rec_e[0:1, :],
                                 start=True, stop=True)
                nc.tensor.matmul(psr[64:128, :], ones1_sb[:], rec_o[0:1, :],
                                 start=True, stop=True)
                nc.vector.tensor_copy(rsb[:], psr[:])
                nc.vector.tensor_tensor(attn_sb[0:64, sl], attn_sb[0:64, sl],
                                        rsb[0:64, :], op=AluOpType.mult)
                nc.vector.tensor_tensor(attn_sb[64:128, sl], attn_sb[64:128, sl],
                                        rsb[64:128, :], op=AluOpType.mult)

            def oproj_unit(tt16, opool):
                psos = [
                    ps4.tile([P, 512], f32, tag="pso", bufs=2, name=f"pso_{tt16}_{mb}")
                    for mb in range(2)
                ]
                for hc in range(4):
                    seg = hc * NTT + tt16 // 4
                    col = (seg * 4 + tt16 % 4) * P
                    for mb in range(2):
                        nc.tensor.matmul(
                            psos[mb][:],
                            attn_sb[:, col: col + P],
                            wo_sb[:, hc * 1024 + mb * 512: hc * 1024 + (mb + 1) * 512],
                            start=(hc == 0),
                            stop=(hc == 3),
                        )
                for mb in range(2):
                    osb = opool.tile([P, 512], bf, tag="osb", bufs=6, name=f"osb_{tt16}_{mb}")
                    nc.vector.tensor_copy(osb[:], psos[mb][:])
                    for dh in range(2):
                        nc.sync.dma_start(
                            outp[tt16 * P:(tt16 + 1) * P,
                                 mb * 512 + dh * 256: mb * 512 + (dh + 1) * 256],
                            osb[:, dh * 256:(dh + 1) * 256],
                        )

            with tc.tile_pool(name="out", bufs=1) as opool:
                def seg_hp_tb(s):
                    # tb-major order: all head pairs finish query-block tb
                    # before tb+1, so each o-proj group becomes ready early
                    # and its units spread over subsequent attention slots.
                    return s % 4, s // 4

                norm_pending = []
                ready_units = []
                normed = set()

                def flush_norm(tail=False):
                    seg, hp, tb = norm_pending.pop(0)
                    norm_seg(seg, tail)
                    normed.add(seg)
                    if all((h * NTT + tb) in normed for h in range(4)):
                        ready_units.extend(range(tb * 4, tb * 4 + 4))

                for j in range(4):
                    s_chunk(0, 0, j)
                slot = 0
                for s in range(16):
                    hp, tb = seg_hp_tb(s)
                    psav = [
                        ps2.tile([P, 512], f32, tag="psav", bufs=2, name=f"psav_{hp}_{tb}_{i}")
                        for i in range(2)
                    ]
                    js_a = list(range(4 * tb + 4))
                    if s + 1 < 16:
                        nhp, ntb = seg_hp_tb(s + 1)
                        js_s = list(range(4 * ntb + 4))
                    else:
                        js_s = []
                    for m in range(max(len(js_s), len(js_a) + SHIFT)):
                        if m < len(js_s):
                            s_chunk(nhp, ntb, js_s[m])
                        am = m - SHIFT
                        if 0 <= am < len(js_a):
                            av_chunk(hp, tb, js_a[am], psav)
                        slot += 1
                        if ready_units and slot % 5 == 0:
                            oproj_unit(ready_units.pop(0), opool)
                    norm_pending.append((hp * NTT + tb, hp, tb))
                    if len(norm_pending) > 2:
                        flush_norm()
                while norm_pending:
                    flush_norm(tail=True)
                while ready_units:
                    oproj_unit(ready_units.pop(0), opool)


_cached = {}


def build_program(split=True, ncopies=1):
    key = ("nc", ncopies)
    if key not in _cached:
        import concourse.bass as bass
        import concourse.tile as tile

        nc = bass.Bass("TRN2", target_bir_lowering=False, debug=False)
        with tile.TileContext(nc) as tc:
            aps = _declare(nc)
            for _ in range(ncopies):
                _emit(nc, tc, aps)
        _cached[key] = nc
    if split and not _cached.get(("split", ncopies)):
        _split_waits(_cached[key])
        _cached[("split", ncopies)] = True
    return _cached[key]


def make_in_maps(x, W_qkv, b_qkv, W_o):
    x = np.asarray(x, dtype=np.float32)
    W_qkv = np.asarray(W_qkv, dtype=np.float32)
    b_qkv = np.asarray(b_qkv, dtype=np.float32)
    W_o = np.asarray(W_o, dtype=np.float32)
    maskT = np.triu(np.ones((P, P), np.float32)).astype(BF16)
    ones1 = np.ones((1, 64), BF16)
    in_maps = []
    for core in range(NCORES):
        b, g = core // 2, core % 2
        qs = slice(g * CQ, (g + 1) * CQ)
        xTc = np.ascontiguousarray(x[b].T)
        wq = W_qkv[:, 0:C][:, qs]
        wk = W_qkv[:, C:2 * C][:, qs]
        wvs = np.ascontiguousarray(W_qkv[:, 2 * C:3 * C][:, qs]).astype(BF16)
        wqk8 = np.ascontiguousarray(
            np.concatenate([wq, wk], axis=1) * WSCALE
        ).astype(E4M3)
        bq = b_qkv[0:C][qs]
        bk = b_qkv[C:2 * C][qs]
        bv = b_qkv[2 * C:3 * C][qs]
        bqk_t = np.ascontiguousarray(
            np.concatenate([bq, bk]).reshape(8, P).T * WSCALE
        ).astype(np.float32)
        bvb = np.ascontiguousarray(
            np.broadcast_to(bv, (P, CQ))
        ).astype(np.float32)
        wos = np.ascontiguousarray(W_o[qs, :]).astype(BF16)
        in_maps.append(
            dict(xT=xTc.astype(BF16), xT8=xTc.astype(E4M3), wqk8=wqk8,
                 bqk=bqk_t, wv=wvs, bvb=bvb, wo=wos, maskT=maskT, ones1=ones1)
        )
    return in_maps


def run(x, W_qkv, b_qkv, W_o, b_o, trace=False, trace_kwargs=None):
    import time as _time

    from concourse.bass_utils import run_bass_kernel_spmd

    nc = build_program()
    in_maps = make_in_maps(x, W_qkv, b_qkv, W_o)
    last_err = None
    for attempt in range(3):
        try:
            res = run_bass_kernel_spmd(
                nc, in_maps, core_ids=list(range(NCORES)), trace=trace,
                **(trace_kwargs or {}),
            )
            break
        except Exception as e:  # transient device wedge -> retry
            last_err = e
            _time.sleep(5)
    else:
        raise last_err
    b_o = np.asarray(b_o, dtype=np.float32)
    out = np.empty((B, T, C), np.float32)
    for b in range(B):
        out[b] = (res.results[2 * b]["outp"].astype(np.float32)
                  + res.results[2 * b + 1]["outp"].astype(np.float32) + b_o)
    return out, res


def kernel(x, W_qkv, b_qkv, W_o, b_o):
    out, _ = run(x, W_qkv, b_qkv, W_o, b_o, trace=False)
    return out
